# revision 5
# baseline (speedup 1.0000x reference)
# Conv2d 3x3 SAME (stride 1) on Trainium2, data-parallel over batch on 8 cores.
#
# Full problem: x[16, 64, 256, 256] f32, weight[128, 64, 3, 3], bias[128]
#   -> out[16, 128, 256, 256] f32.
#
# Per-core kernel (2 images/core): conv lowered to shift-and-matmul, v2.
#
# Roofline (per core): 9 taps x 64ci x 128co x 256x256 x 2img = 9.66 G MAC
#   -> ~247us at the fp16 PE peak (dual-tile, both 64-row halves active).
#   HBM: x fp16 16.8MB + y fp16 33.6MB = 50MB -> 141us at 358 GB/s.
#   So the kernel should be purely PE-bound; v1 measured 311us because it
#   moved 105MB (x loaded twice + y in f32) and saturated HBM end to end.
#
# v2 structure ("strip pair"):
#   - The dual-tile trick runs tap t for TWO independent 16-row strips
#     concurrently: strip A (output rows r0..r0+15) streams from SBUF
#     partitions 0..63 into PE rows 0..63 (tile_position (0,0)), strip B
#     (rows r0+16..r0+31) from partitions 64..127 (tile_position (64,0)).
#     Pairing two *strips* instead of two row-groups of the same strip
#     means each half-strip of x is DMAd once, into one partition half —
#     no duplicated HBM read and no on-chip copy.
#   - Host pre-pads x with the zero border -> xp[bpc, 64, 258, 258] fp16,
#     so a tap (kh, kw) is an AP offset into the SBUF strip; no edge
#     handling on device.
#   - PSUM accumulates the 9 taps per 4-row output group (N = 4*256 =
#     1024 moving elements, 2 PSUM banks); DVE evicts PSUM->SBUF fused
#     with the bias add, converting to fp16.
#   - y is stored fp16 (halves the dominant HBM stream; adds ~5e-4
#     relative error vs the 2e-2 budget) and upcast to f32 on the host.
#   - x loads ride the sync HWDGE ring, y stores the scalar ring.

import numpy as np

import concourse.bass as bass
import concourse.mybir as mybir
import concourse.tile as tile
from concourse import bacc
from concourse.bass_utils import run_bass_kernel_spmd

N_CORES = 8
B, C_IN, H, W = 16, 64, 256, 256
C_OUT = 128
BPC = B // N_CORES  # images per core

F16 = mybir.dt.float16
F32 = mybir.dt.float32

SROWS = 16  # output rows per half-strip (one partition half)


def build_nc(bpc=BPC, h=H, w=W, gr=4):
    """Per-core Bass module. Input xp is the host-padded image
    [bpc, C_IN, h+2, w+2] (zero border), fp16. gr = output rows per PSUM
    accumulation group (N = gr*w moving elements per matmul)."""
    assert h % (2 * SROWS) == 0 and SROWS % gr == 0
    wp = w + 2
    xrows = SROWS + 2  # row slots per half-strip
    nc = bacc.Bacc("TRN2", target_bir_lowering=False, debug=False)

    xp_d = nc.dram_tensor("xp", [bpc, C_IN, h + 2, wp], F16, kind="ExternalInput")
    # all 9 taps, replicated into both partition halves: [2*C_IN, 9, C_OUT]
    wall_d = nc.dram_tensor("wall", [2 * C_IN, 9, C_OUT], F16, kind="ExternalInput")
    bias_d = nc.dram_tensor("bias", [C_OUT, 1], F32, kind="ExternalInput")
    y_d = nc.dram_tensor("y", [bpc, C_OUT, h, w], F16, kind="ExternalOutput")

    with tile.TileContext(nc) as tc:
        with (
            tc.tile_pool(name="consts", bufs=1) as consts,
            tc.tile_pool(name="xpool", bufs=6) as xpool,
            tc.tile_pool(name="ypool", bufs=4) as ypool,
            # a [C_OUT, gr, w] f32 tile is gr/2 PSUM banks; 2 tags (psa, psb)
            # x bufs must fit in 8 banks -> bufs = 8 / gr
            tc.tile_pool(name="psum", bufs=8 // gr, space="PSUM") as psum,
        ):
            wall_sb = consts.tile([2 * C_IN, 9, C_OUT], F16)
            nc.sync.dma_start(out=wall_sb, in_=wall_d.ap())
            bias_sb = consts.tile([C_OUT, 1], F32)
            nc.sync.dma_start(out=bias_sb, in_=bias_d.ap())

            for n in range(bpc):
                for r0 in range(0, h, 2 * SROWS):
                    r1 = r0 + SROWS
                    # slot s of the lower half <-> padded row r0+s; of the
                    # upper half <-> padded row r1+s. Output row j reads
                    # padded rows j..j+2, so group row ja+i tap kh is slot
                    # ja+i+kh — max 12+3+2 = 17 < 18 slots.
                    xl = xpool.tile([128, xrows, wp], F16, tag="xl")
                    nc.sync.dma_start(
                        out=xl[0:C_IN, :, :],
                        in_=xp_d.ap()[n, :, r0 : r0 + xrows, :],
                    )
                    nc.sync.dma_start(
                        out=xl[C_IN:128, :, :],
                        in_=xp_d.ap()[n, :, r1 : r1 + xrows, :],
                    )

                    ylo = ypool.tile([C_OUT, SROWS, w], F16, tag="ylo")
                    yhi = ypool.tile([C_OUT, SROWS, w], F16, tag="yhi")
                    for g in range(SROWS // gr):
                        ja = gr * g
                        psa = psum.tile([C_OUT, gr, w], F32, tag="psa")
                        psb = psum.tile([C_OUT, gr, w], F32, tag="psb")
                        for t in range(9):
                            kh, kw = divmod(t, 3)
                            nc.tensor.matmul(
                                psa,
                                lhsT=wall_sb[0:C_IN, t, :],
                                rhs=xl[0:C_IN, ja + kh : ja + kh + gr, kw : kw + w],
                                start=(t == 0),
                                stop=(t == 8),
                                tile_position=(0, 0),
                            )
                            nc.tensor.matmul(
                                psb,
                                lhsT=wall_sb[C_IN:128, t, :],
                                rhs=xl[C_IN:128, ja + kh : ja + kh + gr, kw : kw + w],
                                start=(t == 0),
                                stop=(t == 8),
                                tile_position=(64, 0),
                            )
                        nc.vector.tensor_scalar_add(ylo[:, ja : ja + gr, :], psa, bias_sb)
                        nc.vector.tensor_scalar_add(yhi[:, ja : ja + gr, :], psb, bias_sb)
                    nc.scalar.dma_start(out=y_d.ap()[n, :, r0 : r0 + SROWS, :], in_=ylo)
                    nc.scalar.dma_start(out=y_d.ap()[n, :, r1 : r1 + SROWS, :], in_=yhi)

    nc.compile()
    return nc


def pad_x(x):
    """[n, c, h, w] -> zero-bordered fp16 [n, c, h+2, w+2]."""
    n, c, h, w = x.shape
    xp = np.zeros((n, c, h + 2, w + 2), np.float16)
    xp[:, :, 1 : h + 1, 1 : w + 1] = x
    return xp


def prep_weights(weight):
    """weight [C_OUT, C_IN, 3, 3] -> lhsT layout [2*ci, tap, co]."""
    wt = np.ascontiguousarray(np.transpose(weight, (1, 2, 3, 0)).astype(np.float16))
    w9 = wt.reshape(C_IN, 9, C_OUT)
    return np.ascontiguousarray(np.concatenate([w9, w9], axis=0))


_NC_CACHE = {}
LAST_RESULT = None  # BassKernelResults of the most recent run (for test harness)
TRACE = False
# gr=4 (N=1024 moving, 2-bank PSUM tiles) fails the walrus ISA check on
# TRN2 — matmul PSUM output must fit one 2KB bank -> gr=2 (N=512).
GR = 2


def kernel(x, weight, bias):
    global LAST_RESULT
    x = np.asarray(x, dtype=np.float32)
    weight = np.asarray(weight, dtype=np.float32)
    bias = np.asarray(bias, dtype=np.float32)

    key = ("v2", GR)
    if key not in _NC_CACHE:
        _NC_CACHE[key] = build_nc(gr=GR)
    nc = _NC_CACHE[key]

    xp = pad_x(x)
    wall = prep_weights(weight)
    bias2 = np.ascontiguousarray(bias.reshape(C_OUT, 1))

    in_maps = []
    for c in range(N_CORES):
        in_maps.append(
            {
                "xp": xp[c * BPC : (c + 1) * BPC],
                "wall": wall,
                "bias": bias2,
            }
        )

    res = run_bass_kernel_spmd(nc, in_maps, core_ids=list(range(N_CORES)), trace=TRACE)
    LAST_RESULT = res
    out = np.concatenate([r["y"] for r in res.results], axis=0).astype(np.float32)
    return out


# revision 7
# speedup vs baseline: 1.1852x; 1.1852x over previous
# Conv2d 3x3 SAME (stride 1) on Trainium2, data-parallel over batch on 8 cores.
#
# Full problem: x[16, 64, 256, 256] f32, weight[128, 64, 3, 3], bias[128]
#   -> out[16, 128, 256, 256] f32.
#
# Per-core kernel (2 images/core): conv lowered to shift-and-matmul, v3.
#
# Roofline (per core): 9 taps x 64ci x 128co x 256x256 x 2img = 9.66 G MAC
#   -> ~246us at the fp16 PE peak (dual-tile, both 64-row halves active).
#   HBM: x fp16 16.8MB + y fp16 33.6MB = 50MB -> ~141us at 358 GB/s.
#   The kernel should therefore be PE-bound. v1 moved 105MB (x read twice,
#   y in f32) and was HBM-bound at 311us; v2 fixed the traffic and ran the
#   PE 100% busy mid-kernel.
#
# Structure ("strip pair"):
#   - The dual-tile trick runs tap t for TWO independent 16-row strips
#     concurrently: strip A (output rows r0..r0+15) streams from SBUF
#     partitions 0..63 into PE rows 0..63 (tile_position (0,0)), strip B
#     (rows r0+16..r0+31) from partitions 64..127 (tile_position (64,0)).
#     Pairing two strips instead of two row-groups of one strip means each
#     half-strip of x is DMAd once, into one partition half — no duplicated
#     HBM read and no on-chip copy.
#   - B processes its groups rotated by +4 relative to A, so the two
#     concurrently-streaming rhs reads always sit at different SBUF byte
#     offsets. v2 ran A and B at identical offsets (different partition
#     halves) and every dual slot paid ~+48ns — same-address port conflict.
#   - Host pre-pads x -> xp[bpc, 64, 258, 258] fp16; a tap (kh, kw) is an
#     AP offset into the SBUF strip, no edge handling on device.
#   - PSUM accumulates 9 taps per 2-row group (N = 512, one bank). PSUM
#     evictions are fused with the bias add and the f32->fp16 convert:
#     psa on DVE (tensor_scalar_add), psb on ScalarE (activation Identity
#     with per-partition bias) so neither engine rides the critical path.
#   - y is stored fp16 (halves the dominant HBM stream; adds ~5e-4 rel
#     error vs the 2e-2 budget) and upcast to f32 on the host. ylo rides
#     the scalar HWDGE ring, yhi the sync ring, x loads the sync ring.
#   - A handful of warm-up matmuls run while the first x strips are in
#     flight so the PE HAM clock-gate (cold 1.2 GHz -> warm 2.4 GHz after
#     ~3.4us of sustained activity) is already released when real work
#     starts.

import numpy as np

import concourse.bass as bass
import concourse.mybir as mybir
import concourse.tile as tile
from concourse import bacc
from concourse.bass_utils import run_bass_kernel_spmd

N_CORES = 8
B, C_IN, H, W = 16, 64, 256, 256
C_OUT = 128
BPC = B // N_CORES  # images per core

F16 = mybir.dt.float16
F32 = mybir.dt.float32

SROWS = 16  # output rows per half-strip (one partition half)
GR = 2  # output rows per PSUM group (N = GR*W = 512; one 2KB bank)
N_WARMUP = 12  # PE warm-up matmuls issued while the first strips load


def build_nc(bpc=BPC, h=H, w=W, gr=GR):
    """Per-core Bass module. Input xp is the host-padded image
    [bpc, C_IN, h+2, w+2] (zero border), fp16."""
    assert h % (2 * SROWS) == 0 and SROWS % gr == 0
    ng = SROWS // gr  # groups per half-strip
    wp = w + 2
    xrows = SROWS + 2  # row slots per half-strip
    nc = bacc.Bacc("TRN2", target_bir_lowering=False, debug=False)

    xp_d = nc.dram_tensor("xp", [bpc, C_IN, h + 2, wp], F16, kind="ExternalInput")
    # all 9 taps, replicated into both partition halves: [2*C_IN, 9, C_OUT]
    wall_d = nc.dram_tensor("wall", [2 * C_IN, 9, C_OUT], F16, kind="ExternalInput")
    bias_d = nc.dram_tensor("bias", [C_OUT, 1], F32, kind="ExternalInput")
    y_d = nc.dram_tensor("y", [bpc, C_OUT, h, w], F16, kind="ExternalOutput")

    with tile.TileContext(nc) as tc:
        with (
            tc.tile_pool(name="consts", bufs=1) as consts,
            tc.tile_pool(name="xpool", bufs=2) as xpool,
            tc.tile_pool(name="ypool", bufs=4) as ypool,
            # psa/psb tags x 3 bufs = 6 banks, + 1 warm-up bank = 7 of 8
            tc.tile_pool(name="psum", bufs=3, space="PSUM") as psum,
            tc.tile_pool(name="warm", bufs=1, space="PSUM") as warm,
        ):
            wall_sb = consts.tile([2 * C_IN, 9, C_OUT], F16)
            nc.sync.dma_start(out=wall_sb, in_=wall_d.ap())
            bias_sb = consts.tile([C_OUT, 1], F32)
            nc.sync.dma_start(out=bias_sb, in_=bias_d.ap())

            # Warm-up: dummy matmuls against the (small, first-landing)
            # weight tile keep the PE busy while the first x strips are
            # still in flight, releasing the HAM throttle early. Result is
            # never read.
            wps = warm.tile([C_OUT, gr * w], F32, tag="warm")
            for i in range(N_WARMUP):
                nc.tensor.matmul(
                    wps,
                    lhsT=wall_sb[0:C_IN, 0, :],
                    rhs=wall_sb[0:C_IN, 0 : (gr * w) // C_OUT, :],
                    start=True,
                    stop=True,
                    tile_position=(0, 0),
                )

            for n in range(bpc):
                for r0 in range(0, h, 2 * SROWS):
                    r1 = r0 + SROWS
                    # slot s of the lower half <-> padded row r0+s; of the
                    # upper half <-> padded row r1+s. Output row j reads
                    # padded rows j..j+2, so group row ja+i tap kh is slot
                    # ja+i+kh — max 14+1+2 = 17 < 18 slots.
                    xl = xpool.tile([128, xrows, wp], F16, tag="xl")
                    nc.sync.dma_start(
                        out=xl[0:C_IN, :, :],
                        in_=xp_d.ap()[n, :, r0 : r0 + xrows, :],
                    )
                    nc.sync.dma_start(
                        out=xl[C_IN:128, :, :],
                        in_=xp_d.ap()[n, :, r1 : r1 + xrows, :],
                    )

                    ylo = ypool.tile([C_OUT, SROWS, w], F16, tag="ylo")
                    yhi = ypool.tile([C_OUT, SROWS, w], F16, tag="yhi")
                    for g in range(ng):
                        ja = gr * g
                        jb = gr * ((g + ng // 2) % ng)  # B rotated: offsets differ
                        psa = psum.tile([C_OUT, gr, w], F32, tag="psa")
                        psb = psum.tile([C_OUT, gr, w], F32, tag="psb")
                        for t in range(9):
                            kh, kw = divmod(t, 3)
                            nc.tensor.matmul(
                                psa,
                                lhsT=wall_sb[0:C_IN, t, :],
                                rhs=xl[0:C_IN, ja + kh : ja + kh + gr, kw : kw + w],
                                start=(t == 0),
                                stop=(t == 8),
                                tile_position=(0, 0),
                            )
                            nc.tensor.matmul(
                                psb,
                                lhsT=wall_sb[C_IN:128, t, :],
                                rhs=xl[C_IN:128, jb + kh : jb + kh + gr, kw : kw + w],
                                start=(t == 0),
                                stop=(t == 8),
                                tile_position=(64, 0),
                            )
                        nc.vector.tensor_scalar_add(ylo[:, ja : ja + gr, :], psa, bias_sb)
                        nc.scalar.activation(
                            yhi[:, jb : jb + gr, :],
                            psb,
                            mybir.ActivationFunctionType.Identity,
                            bias=bias_sb,
                        )
                    nc.scalar.dma_start(out=y_d.ap()[n, :, r0 : r0 + SROWS, :], in_=ylo)
                    nc.sync.dma_start(out=y_d.ap()[n, :, r1 : r1 + SROWS, :], in_=yhi)

    nc.compile()
    return nc


def pad_x(x):
    """[n, c, h, w] -> zero-bordered fp16 [n, c, h+2, w+2]."""
    n, c, h, w = x.shape
    xp = np.zeros((n, c, h + 2, w + 2), np.float16)
    xp[:, :, 1 : h + 1, 1 : w + 1] = x
    return xp


def prep_weights(weight):
    """weight [C_OUT, C_IN, 3, 3] -> lhsT layout [2*ci, tap, co]."""
    wt = np.ascontiguousarray(np.transpose(weight, (1, 2, 3, 0)).astype(np.float16))
    w9 = wt.reshape(C_IN, 9, C_OUT)
    return np.ascontiguousarray(np.concatenate([w9, w9], axis=0))


_NC_CACHE = {}
LAST_RESULT = None  # BassKernelResults of the most recent run (for test harness)
TRACE = False


def kernel(x, weight, bias):
    global LAST_RESULT
    x = np.asarray(x, dtype=np.float32)
    weight = np.asarray(weight, dtype=np.float32)
    bias = np.asarray(bias, dtype=np.float32)

    key = ("v3", GR)
    if key not in _NC_CACHE:
        _NC_CACHE[key] = build_nc()
    nc = _NC_CACHE[key]

    xp = pad_x(x)
    wall = prep_weights(weight)
    bias2 = np.ascontiguousarray(bias.reshape(C_OUT, 1))

    in_maps = []
    for c in range(N_CORES):
        in_maps.append(
            {
                "xp": xp[c * BPC : (c + 1) * BPC],
                "wall": wall,
                "bias": bias2,
            }
        )

    res = run_bass_kernel_spmd(nc, in_maps, core_ids=list(range(N_CORES)), trace=TRACE)
    LAST_RESULT = res
    out = np.concatenate([r["y"] for r in res.results], axis=0).astype(np.float32)
    return out


# revision 12
# speedup vs baseline: 1.1994x; 1.0120x over previous
# Conv2d 3x3 SAME (stride 1) on Trainium2, data-parallel over batch on 8 cores.
#
# Full problem: x[16, 64, 256, 256] f32, weight[128, 64, 3, 3], bias[128]
#   -> out[16, 128, 256, 256] f32.
#
# Per-core kernel (2 images/core): conv lowered to shift-and-matmul, v3.
#
# Roofline (per core): 9 taps x 64ci x 128co x 256x256 x 2img = 9.66 G MAC
#   -> ~246us at the fp16 PE peak (dual-tile, both 64-row halves active).
#   HBM: x fp16 16.8MB + y fp16 33.6MB = 50MB -> ~141us at 358 GB/s.
#   The kernel should therefore be PE-bound. v1 moved 105MB (x read twice,
#   y in f32) and was HBM-bound at 311us; v2 fixed the traffic and ran the
#   PE 100% busy mid-kernel.
#
# Structure ("strip pair"):
#   - The dual-tile trick runs tap t for TWO independent 16-row strips
#     concurrently: strip A (output rows r0..r0+15) streams from SBUF
#     partitions 0..63 into PE rows 0..63 (tile_position (0,0)), strip B
#     (rows r0+16..r0+31) from partitions 64..127 (tile_position (64,0)).
#     Pairing two strips instead of two row-groups of one strip means each
#     half-strip of x is DMAd once, into one partition half — no duplicated
#     HBM read and no on-chip copy.
#   - B processes its groups rotated by +4 relative to A, so the two
#     concurrently-streaming rhs reads always sit at different SBUF byte
#     offsets. v2 ran A and B at identical offsets (different partition
#     halves) and every dual slot paid ~+48ns — same-address port conflict.
#   - Host pre-pads x -> xp[bpc, 64, 258, 258] fp16; a tap (kh, kw) is an
#     AP offset into the SBUF strip, no edge handling on device.
#   - PSUM accumulates 9 taps per 2-row group (N = 512, one bank). PSUM
#     evictions are fused with the bias add and the f32->fp16 convert:
#     psa on DVE (tensor_scalar_add), psb on ScalarE (activation Identity
#     with per-partition bias) so neither engine rides the critical path.
#   - y is stored fp16 (halves the dominant HBM stream; adds ~5e-4 rel
#     error vs the 2e-2 budget) and upcast to f32 on the host. ylo rides
#     the scalar HWDGE ring, yhi the sync ring, x loads the sync ring.
#   - A handful of warm-up matmuls run while the first x strips are in
#     flight so the PE HAM clock-gate (cold 1.2 GHz -> warm 2.4 GHz after
#     ~3.4us of sustained activity) is already released when real work
#     starts.

import numpy as np

import concourse.bass as bass
import concourse.mybir as mybir
import concourse.tile as tile
from concourse import bacc
from concourse.bass_utils import run_bass_kernel_spmd

N_CORES = 8
B, C_IN, H, W = 16, 64, 256, 256
C_OUT = 128
BPC = B // N_CORES  # images per core

F16 = mybir.dt.float16
F32 = mybir.dt.float32

SROWS = 16  # output rows per half-strip (one partition half)
GR = 2  # output rows per PSUM group (N = GR*W = 512; one 2KB bank)
N_WARMUP = 18  # enough dual pairs to sustain the ~3.4us HAM window


def build_nc(bpc=BPC, h=H, w=W, gr=GR):
    """Per-core Bass module. Input xp is the host-padded image
    [bpc, C_IN, h+2, w+2] (zero border), fp16."""
    assert h % (2 * SROWS) == 0 and SROWS % gr == 0
    ng = SROWS // gr  # groups per half-strip
    wp = w + 2
    xrows = SROWS + 2  # row slots per half-strip
    nc = bacc.Bacc("TRN2", target_bir_lowering=False, debug=False)

    xp_d = nc.dram_tensor("xp", [bpc, C_IN, h + 2, wp], F16, kind="ExternalInput")
    # all 9 taps, replicated into both partition halves: [2*C_IN, 9, C_OUT]
    wall_d = nc.dram_tensor("wall", [2 * C_IN, 9, C_OUT], F16, kind="ExternalInput")
    bias_d = nc.dram_tensor("bias", [C_OUT, 1], F32, kind="ExternalInput")
    y_d = nc.dram_tensor("y", [bpc, C_OUT, h, w], F16, kind="ExternalOutput")

    with tile.TileContext(nc) as tc:
        with (
            tc.tile_pool(name="consts", bufs=1) as consts,
            tc.tile_pool(name="xpool", bufs=2) as xpool,
            tc.tile_pool(name="ypool", bufs=4) as ypool,
            # psa/psb tags x 3 bufs = 6 banks, + 2 warm-up banks = 8 of 8
            tc.tile_pool(name="psum", bufs=3, space="PSUM") as psum,
            tc.tile_pool(name="warm", bufs=1, space="PSUM") as warm,
        ):
            wall_sb = consts.tile([2 * C_IN, 9, C_OUT], F16)
            nc.sync.dma_start(out=wall_sb, in_=wall_d.ap())
            bias_sb = consts.tile([C_OUT, 1], F32)
            nc.sync.dma_start(out=bias_sb, in_=bias_d.ap())

            # Warm-up: dummy matmuls on a memset scratch tile (no DMA
            # dependency) keep the PE busy from the very start, releasing
            # the HAM throttle (cold 1.2 GHz -> warm 2.4 GHz after ~3.4us
            # sustained) before the first real matmul. They must alternate
            # two independent PSUM tiles on the two array halves to issue
            # back-to-back — a single accumulation target serializes on
            # WAW and the resulting drain gaps never sustain the HAM
            # window. Results never read.
            scratch = consts.tile([128, gr * w], F16)
            nc.gpsimd.memset(scratch[:, :], 0)
            wps_a = warm.tile([C_OUT, gr * w], F32, tag="warm_a")
            wps_b = warm.tile([C_OUT, gr * w], F32, tag="warm_b")
            for i in range(N_WARMUP):
                nc.tensor.matmul(
                    wps_a if i % 2 == 0 else wps_b,
                    lhsT=scratch[0:C_IN, 0:C_OUT] if i % 2 == 0 else scratch[C_IN:128, 0:C_OUT],
                    rhs=scratch[0:C_IN, :] if i % 2 == 0 else scratch[C_IN:128, :],
                    start=True,
                    stop=True,
                    tile_position=(0, 0) if i % 2 == 0 else (64, 0),
                )

            for n in range(bpc):
                for r0 in range(0, h, 2 * SROWS):
                    r1 = r0 + SROWS
                    # slot s of the lower half <-> padded row r0+s; of the
                    # upper half <-> padded row r1+s. Output row j reads
                    # padded rows j..j+2, so group row ja+i tap kh is slot
                    # ja+i+kh — max 14+1+2 = 17 < 18 slots.
                    xl = xpool.tile([128, xrows, wp], F16, tag="xl")
                    nc.sync.dma_start(
                        out=xl[0:C_IN, :, :],
                        in_=xp_d.ap()[n, :, r0 : r0 + xrows, :],
                    )
                    nc.sync.dma_start(
                        out=xl[C_IN:128, :, :],
                        in_=xp_d.ap()[n, :, r1 : r1 + xrows, :],
                    )

                    ylo = ypool.tile([C_OUT, SROWS, w], F16, tag="ylo")
                    yhi = ypool.tile([C_OUT, SROWS, w], F16, tag="yhi")
                    half = SROWS // 2
                    for g in range(ng):
                        ja = gr * g
                        jb = gr * ((g + ng // 2) % ng)  # B rotated: offsets differ
                        psa = psum.tile([C_OUT, gr, w], F32, tag="psa")
                        psb = psum.tile([C_OUT, gr, w], F32, tag="psb")
                        for t in range(9):
                            kh, kw = divmod(t, 3)
                            nc.tensor.matmul(
                                psa,
                                lhsT=wall_sb[0:C_IN, t, :],
                                rhs=xl[0:C_IN, ja + kh : ja + kh + gr, kw : kw + w],
                                start=(t == 0),
                                stop=(t == 8),
                                tile_position=(0, 0),
                            )
                            nc.tensor.matmul(
                                psb,
                                lhsT=wall_sb[C_IN:128, t, :],
                                rhs=xl[C_IN:128, jb + kh : jb + kh + gr, kw : kw + w],
                                start=(t == 0),
                                stop=(t == 8),
                                tile_position=(64, 0),
                            )
                        nc.vector.tensor_scalar_add(ylo[:, ja : ja + gr, :], psa, bias_sb)
                        nc.scalar.activation(
                            yhi[:, jb : jb + gr, :],
                            psb,
                            mybir.ActivationFunctionType.Identity,
                            bias=bias_sb,
                        )
                        # store each finished 8-row half as soon as its last
                        # eviction lands: A fills ylo rows in order, B fills
                        # yhi rows 8..16 first (rotation), then 0..8.
                        if g == ng // 2 - 1:
                            nc.scalar.dma_start(
                                out=y_d.ap()[n, :, r0 : r0 + half, :],
                                in_=ylo[:, 0:half, :],
                            )
                            nc.sync.dma_start(
                                out=y_d.ap()[n, :, r1 + half : r1 + SROWS, :],
                                in_=yhi[:, half:SROWS, :],
                            )
                        elif g == ng - 1:
                            nc.scalar.dma_start(
                                out=y_d.ap()[n, :, r0 + half : r0 + SROWS, :],
                                in_=ylo[:, half:SROWS, :],
                            )
                            nc.sync.dma_start(
                                out=y_d.ap()[n, :, r1 : r1 + half, :],
                                in_=yhi[:, 0:half, :],
                            )

    nc.compile()
    return nc


def pad_x(x):
    """[n, c, h, w] -> zero-bordered fp16 [n, c, h+2, w+2]."""
    n, c, h, w = x.shape
    xp = np.zeros((n, c, h + 2, w + 2), np.float16)
    xp[:, :, 1 : h + 1, 1 : w + 1] = x
    return xp


def prep_weights(weight):
    """weight [C_OUT, C_IN, 3, 3] -> lhsT layout [2*ci, tap, co]."""
    wt = np.ascontiguousarray(np.transpose(weight, (1, 2, 3, 0)).astype(np.float16))
    w9 = wt.reshape(C_IN, 9, C_OUT)
    return np.ascontiguousarray(np.concatenate([w9, w9], axis=0))


_NC_CACHE = {}
LAST_RESULT = None  # BassKernelResults of the most recent run (for test harness)
TRACE = False


def kernel(x, weight, bias):
    global LAST_RESULT
    x = np.asarray(x, dtype=np.float32)
    weight = np.asarray(weight, dtype=np.float32)
    bias = np.asarray(bias, dtype=np.float32)

    key = ("v3", GR)
    if key not in _NC_CACHE:
        _NC_CACHE[key] = build_nc()
    nc = _NC_CACHE[key]

    xp = pad_x(x)
    wall = prep_weights(weight)
    bias2 = np.ascontiguousarray(bias.reshape(C_OUT, 1))

    in_maps = []
    for c in range(N_CORES):
        in_maps.append(
            {
                "xp": xp[c * BPC : (c + 1) * BPC],
                "wall": wall,
                "bias": bias2,
            }
        )

    res = run_bass_kernel_spmd(nc, in_maps, core_ids=list(range(N_CORES)), trace=TRACE)
    LAST_RESULT = res
    out = np.concatenate([r["y"] for r in res.results], axis=0).astype(np.float32)
    return out


# revision 13
# speedup vs baseline: 1.2062x; 1.0057x over previous
# Conv2d 3x3 SAME (stride 1) on Trainium2, data-parallel over batch on 8 cores.
#
# Full problem: x[16, 64, 256, 256] f32, weight[128, 64, 3, 3], bias[128]
#   -> out[16, 128, 256, 256] f32.
#
# Per-core kernel (2 images/core): conv lowered to shift-and-matmul, v3.
#
# Roofline (per core): 9 taps x 64ci x 128co x 256x256 x 2img = 9.66 G MAC
#   -> ~246us at the fp16 PE peak (dual-tile, both 64-row halves active).
#   HBM: x fp16 16.8MB + y fp16 33.6MB = 50MB -> ~141us at 358 GB/s.
#   The kernel should therefore be PE-bound. v1 moved 105MB (x read twice,
#   y in f32) and was HBM-bound at 311us; v2 fixed the traffic and ran the
#   PE 100% busy mid-kernel.
#
# Structure ("strip pair"):
#   - The dual-tile trick runs tap t for TWO independent 16-row strips
#     concurrently: strip A (output rows r0..r0+15) streams from SBUF
#     partitions 0..63 into PE rows 0..63 (tile_position (0,0)), strip B
#     (rows r0+16..r0+31) from partitions 64..127 (tile_position (64,0)).
#     Pairing two strips instead of two row-groups of one strip means each
#     half-strip of x is DMAd once, into one partition half — no duplicated
#     HBM read and no on-chip copy.
#   - B processes its groups rotated by +4 relative to A, so the two
#     concurrently-streaming rhs reads always sit at different SBUF byte
#     offsets. v2 ran A and B at identical offsets (different partition
#     halves) and every dual slot paid ~+48ns — same-address port conflict.
#   - Host pre-pads x -> xp[bpc, 64, 258, 258] fp16; a tap (kh, kw) is an
#     AP offset into the SBUF strip, no edge handling on device.
#   - PSUM accumulates 9 taps per 2-row group (N = 512, one bank). PSUM
#     evictions are fused with the bias add and the f32->fp16 convert:
#     psa on DVE (tensor_scalar_add), psb on ScalarE (activation Identity
#     with per-partition bias) so neither engine rides the critical path.
#   - y is stored fp16 (halves the dominant HBM stream; adds ~5e-4 rel
#     error vs the 2e-2 budget) and upcast to f32 on the host. ylo rides
#     the scalar HWDGE ring, yhi the sync ring, x loads the sync ring.
#   - A handful of warm-up matmuls run while the first x strips are in
#     flight so the PE HAM clock-gate (cold 1.2 GHz -> warm 2.4 GHz after
#     ~3.4us of sustained activity) is already released when real work
#     starts.

import numpy as np

import concourse.bass as bass
import concourse.mybir as mybir
import concourse.tile as tile
from concourse import bacc
from concourse.bass_utils import run_bass_kernel_spmd

N_CORES = 8
B, C_IN, H, W = 16, 64, 256, 256
C_OUT = 128
BPC = B // N_CORES  # images per core

F16 = mybir.dt.float16
F32 = mybir.dt.float32

SROWS = 16  # output rows per half-strip (one partition half)
GR = 2  # output rows per PSUM group (N = GR*W = 512; one 2KB bank)
N_WARMUP = 34  # bridge PE activity from preamble end (~7.8us) until
# the first strip's both halves have landed (~14-15us), so the HAM
# throttle releases before real work and never re-arms


def build_nc(bpc=BPC, h=H, w=W, gr=GR):
    """Per-core Bass module. Input xp is the host-padded image
    [bpc, C_IN, h+2, w+2] (zero border), fp16."""
    assert h % (2 * SROWS) == 0 and SROWS % gr == 0
    ng = SROWS // gr  # groups per half-strip
    wp = w + 2
    xrows = SROWS + 2  # row slots per half-strip
    nc = bacc.Bacc("TRN2", target_bir_lowering=False, debug=False)

    xp_d = nc.dram_tensor("xp", [bpc, C_IN, h + 2, wp], F16, kind="ExternalInput")
    # all 9 taps, replicated into both partition halves: [2*C_IN, 9, C_OUT]
    wall_d = nc.dram_tensor("wall", [2 * C_IN, 9, C_OUT], F16, kind="ExternalInput")
    bias_d = nc.dram_tensor("bias", [C_OUT, 1], F32, kind="ExternalInput")
    y_d = nc.dram_tensor("y", [bpc, C_OUT, h, w], F16, kind="ExternalOutput")

    with tile.TileContext(nc) as tc:
        with (
            tc.tile_pool(name="consts", bufs=1) as consts,
            tc.tile_pool(name="xpool", bufs=2) as xpool,
            tc.tile_pool(name="ypool", bufs=4) as ypool,
            # psa/psb tags x 3 bufs = 6 banks, + 2 warm-up banks = 8 of 8
            tc.tile_pool(name="psum", bufs=3, space="PSUM") as psum,
            tc.tile_pool(name="warm", bufs=1, space="PSUM") as warm,
        ):
            wall_sb = consts.tile([2 * C_IN, 9, C_OUT], F16)
            nc.sync.dma_start(out=wall_sb, in_=wall_d.ap())
            bias_sb = consts.tile([C_OUT, 1], F32)
            nc.sync.dma_start(out=bias_sb, in_=bias_d.ap())

            # Warm-up: dummy matmuls on a memset scratch tile (no DMA
            # dependency) keep the PE busy from the very start, releasing
            # the HAM throttle (cold 1.2 GHz -> warm 2.4 GHz after ~3.4us
            # sustained) before the first real matmul. They must alternate
            # two independent PSUM tiles on the two array halves to issue
            # back-to-back — a single accumulation target serializes on
            # WAW and the resulting drain gaps never sustain the HAM
            # window. Results never read.
            scratch = consts.tile([128, gr * w], F16)
            nc.gpsimd.memset(scratch[:, :], 0)
            wps_a = warm.tile([C_OUT, gr * w], F32, tag="warm_a")
            wps_b = warm.tile([C_OUT, gr * w], F32, tag="warm_b")
            for i in range(N_WARMUP):
                nc.tensor.matmul(
                    wps_a if i % 2 == 0 else wps_b,
                    lhsT=scratch[0:C_IN, 0:C_OUT] if i % 2 == 0 else scratch[C_IN:128, 0:C_OUT],
                    rhs=scratch[0:C_IN, :] if i % 2 == 0 else scratch[C_IN:128, :],
                    start=True,
                    stop=True,
                    tile_position=(0, 0) if i % 2 == 0 else (64, 0),
                )

            for n in range(bpc):
                for r0 in range(0, h, 2 * SROWS):
                    r1 = r0 + SROWS
                    # slot s of the lower half <-> padded row r0+s; of the
                    # upper half <-> padded row r1+s. Output row j reads
                    # padded rows j..j+2, so group row ja+i tap kh is slot
                    # ja+i+kh — max 14+1+2 = 17 < 18 slots.
                    xl = xpool.tile([128, xrows, wp], F16, tag="xl")
                    nc.sync.dma_start(
                        out=xl[0:C_IN, :, :],
                        in_=xp_d.ap()[n, :, r0 : r0 + xrows, :],
                    )
                    nc.sync.dma_start(
                        out=xl[C_IN:128, :, :],
                        in_=xp_d.ap()[n, :, r1 : r1 + xrows, :],
                    )

                    ylo = ypool.tile([C_OUT, SROWS, w], F16, tag="ylo")
                    yhi = ypool.tile([C_OUT, SROWS, w], F16, tag="yhi")
                    half = SROWS // 2
                    for g in range(ng):
                        ja = gr * g
                        jb = gr * ((g + ng // 2) % ng)  # B rotated: offsets differ
                        psa = psum.tile([C_OUT, gr, w], F32, tag="psa")
                        psb = psum.tile([C_OUT, gr, w], F32, tag="psb")
                        for t in range(9):
                            kh, kw = divmod(t, 3)
                            nc.tensor.matmul(
                                psa,
                                lhsT=wall_sb[0:C_IN, t, :],
                                rhs=xl[0:C_IN, ja + kh : ja + kh + gr, kw : kw + w],
                                start=(t == 0),
                                stop=(t == 8),
                                tile_position=(0, 0),
                            )
                            nc.tensor.matmul(
                                psb,
                                lhsT=wall_sb[C_IN:128, t, :],
                                rhs=xl[C_IN:128, jb + kh : jb + kh + gr, kw : kw + w],
                                start=(t == 0),
                                stop=(t == 8),
                                tile_position=(64, 0),
                            )
                        nc.vector.tensor_scalar_add(ylo[:, ja : ja + gr, :], psa, bias_sb)
                        nc.scalar.activation(
                            yhi[:, jb : jb + gr, :],
                            psb,
                            mybir.ActivationFunctionType.Identity,
                            bias=bias_sb,
                        )
                        # store each finished 8-row half as soon as its last
                        # eviction lands: A fills ylo rows in order, B fills
                        # yhi rows 8..16 first (rotation), then 0..8.
                        if g == ng // 2 - 1:
                            nc.scalar.dma_start(
                                out=y_d.ap()[n, :, r0 : r0 + half, :],
                                in_=ylo[:, 0:half, :],
                            )
                            nc.sync.dma_start(
                                out=y_d.ap()[n, :, r1 + half : r1 + SROWS, :],
                                in_=yhi[:, half:SROWS, :],
                            )
                        elif g == ng - 1:
                            nc.scalar.dma_start(
                                out=y_d.ap()[n, :, r0 + half : r0 + SROWS, :],
                                in_=ylo[:, half:SROWS, :],
                            )
                            nc.sync.dma_start(
                                out=y_d.ap()[n, :, r1 : r1 + half, :],
                                in_=yhi[:, 0:half, :],
                            )

    nc.compile()
    return nc


def pad_x(x):
    """[n, c, h, w] -> zero-bordered fp16 [n, c, h+2, w+2]."""
    n, c, h, w = x.shape
    xp = np.zeros((n, c, h + 2, w + 2), np.float16)
    xp[:, :, 1 : h + 1, 1 : w + 1] = x
    return xp


def prep_weights(weight):
    """weight [C_OUT, C_IN, 3, 3] -> lhsT layout [2*ci, tap, co]."""
    wt = np.ascontiguousarray(np.transpose(weight, (1, 2, 3, 0)).astype(np.float16))
    w9 = wt.reshape(C_IN, 9, C_OUT)
    return np.ascontiguousarray(np.concatenate([w9, w9], axis=0))


_NC_CACHE = {}
LAST_RESULT = None  # BassKernelResults of the most recent run (for test harness)
TRACE = False


def kernel(x, weight, bias):
    global LAST_RESULT
    x = np.asarray(x, dtype=np.float32)
    weight = np.asarray(weight, dtype=np.float32)
    bias = np.asarray(bias, dtype=np.float32)

    key = ("v3", GR)
    if key not in _NC_CACHE:
        _NC_CACHE[key] = build_nc()
    nc = _NC_CACHE[key]

    xp = pad_x(x)
    wall = prep_weights(weight)
    bias2 = np.ascontiguousarray(bias.reshape(C_OUT, 1))

    in_maps = []
    for c in range(N_CORES):
        in_maps.append(
            {
                "xp": xp[c * BPC : (c + 1) * BPC],
                "wall": wall,
                "bias": bias2,
            }
        )

    res = run_bass_kernel_spmd(nc, in_maps, core_ids=list(range(N_CORES)), trace=TRACE)
    LAST_RESULT = res
    out = np.concatenate([r["y"] for r in res.results], axis=0).astype(np.float32)
    return out
